# revision 1
# baseline (speedup 1.0000x reference)
"""CropAndResize (tf.image.crop_and_resize semantics, bilinear, extrap=0)
Trainium2 Bass kernel, data-parallel over 8 NeuronCores.

Full inputs:  img (4,512,64,64) f32, rois (4,300,4) f32, input_image (4,3,1024,1024) f32
Full output:  (4,300,512,7,7) f32

Sharding: core c handles image n = c//2 and that image's roi slice
[ (c%2)*150 : (c%2)*150+150 ] (padded to 160 = 10 batches of 16).

Per-core device program (fp16 compute, f32 in/out):
  1. img NCHW f32 -> SBUF -> cast fp16 -> xbar DMA-transpose into the
     gather layout img_g[p, j, c] = row(hw=j*128+p), 1024B/row.
  2. RoI prep on DVE in [49pt, 160roi] layout: sample coords, clipped
     floors, lerp weights folded with the validity mask, gather indices.
  3. Index wrap into the dma_gather int16 [16-wrapped, replicated] layout
     and weight flatten to a per-corner row, via small DMAs.
  4. Per batch (16 rois): 4 SBUF-source transpose-mode dma_gathers
     (channels land on partitions), PE ones-matmul broadcast of weights,
     DVE lerp: out = sum_k T_k * W_k, write f32 out with strided DMA.
"""

import os
import sys

import numpy as np

_RL_REPO_CANDIDATES = ["/opt/trn_rl_repo", "/root/.axon_site/_ro/trn_rl_repo"]
for _p in _RL_REPO_CANDIDATES:
    if os.path.isdir(_p) and _p not in sys.path:
        sys.path.insert(0, _p)

import ml_dtypes  # noqa: E402

# ---------------------------------------------------------------- constants
N_CORES = 8
N, C, H, W = 4, 512, 64, 64
B = 300
POOL = 7
PTS = POOL * POOL  # 49
IH, IW = 1024.0, 1024.0
R_CORE = B // 2          # 150 real rois per core
R_PAD = 160              # padded roi count per core
RB = 8                   # rois per batch
NBATCH = R_PAD // RB     # 10
SLOT = 64                # per-roi slot stride in the gather index space
NIDX = RB * SLOT         # 1024 gather indices per batch (mult of 128)
SPB = NIDX // 16         # 64 wrapped-idx free slots per batch
HW = H * W               # 4096

_prog_cache = {}


def _build_program():
    import concourse.bass as bass
    import concourse.bacc as bacc
    import concourse.mybir as mybir
    import concourse.tile as tile

    f32 = mybir.dt.float32
    f16 = mybir.dt.float16
    i16 = mybir.dt.int16
    Alu = mybir.AluOpType

    nc = bacc.Bacc("TRN2", target_bir_lowering=False, debug=False,
                   num_devices=N_CORES)

    img_in = nc.dram_tensor("img", (C, HW), f32, kind="ExternalInput")
    rois_in = nc.dram_tensor("rois", (R_PAD, 4), f32, kind="ExternalInput")
    consts_in = nc.dram_tensor("consts", (1, 256), f32, kind="ExternalInput")
    out_t = nc.dram_tensor("out", (R_CORE, C, POOL, POOL), f32,
                           kind="ExternalOutput")

    with tile.TileContext(nc) as tc:
        _body(tc, nc, bass, mybir, tile, img_in, rois_in, consts_in, out_t,
              f32, f16, i16, Alu)

    nc.compile()
    return nc


def _body(tc, nc, bass, mybir, tile, img_in, rois_in, consts_in, out_t,
          f32, f16, i16, Alu):
    from contextlib import ExitStack
    ctx = ExitStack()
    with ctx:
        import os as _os
        GB = int(_os.environ.get("K_GBUFS", "3"))
        WB = int(_os.environ.get("K_WBUFS", "3"))
        OB = int(_os.environ.get("K_OBUFS", "2"))
        PB = int(_os.environ.get("K_PBUFS", "4"))
        const_pool = ctx.enter_context(tc.tile_pool(name="const", bufs=1))
        prep_pool = ctx.enter_context(tc.tile_pool(name="prep", bufs=1))
        imgstage = ctx.enter_context(tc.tile_pool(name="imgstage", bufs=2))
        gather_pool = ctx.enter_context(tc.tile_pool(name="gather", bufs=GB))
        w_pool = ctx.enter_context(tc.tile_pool(name="wts", bufs=WB))
        wrow_pool = ctx.enter_context(tc.tile_pool(name="wrow", bufs=WB))
        o_pool = ctx.enter_context(tc.tile_pool(name="outs", bufs=OB))
        dram_pool = ctx.enter_context(
            tc.tile_pool(name="dram", bufs=1, space="DRAM"))
        psum_pool = ctx.enter_context(
            tc.tile_pool(name="psum", bufs=PB, space="PSUM"))

        # ------------------------------------------------ constants
        # consts row: [0:49]=g_y per pt, [49:98]=g_x per pt, [98:226]=ones
        gy_col = const_pool.tile([PTS, 1], f32, tag="gy")
        gx_col = const_pool.tile([PTS, 1], f32, tag="gx")
        # strided loads: partition p <- consts[0, p] / consts[0, 49+p]
        nc.sync.dma_start(gy_col[:, :], consts_in.ap()[0:1, 0:PTS].rearrange(
            "a p -> p a"))
        nc.sync.dma_start(gx_col[:, :], consts_in.ap()[0:1, PTS:2 * PTS]
                          .rearrange("a p -> p a"))
        ones_f32 = const_pool.tile([1, 128], f32, tag="ones32")
        nc.sync.dma_start(ones_f32[:, :], consts_in.ap()[0:1, 98:226])
        ones16 = const_pool.tile([1, 128], f16, tag="ones16")
        nc.vector.tensor_copy(ones16[:, :], ones_f32[:, :])

        # ------------------------------------------------ image prep
        # img_nhwc[hw, c] fp16 in DRAM; built via SBUF xbar transpose:
        # img_g[p, j, c] = img row hw=j*128+p -> DRAM row-major write
        img_nhwc = dram_pool.tile([HW, C], f16, name="img_nhwc")
        img_g = const_pool.tile([128, 32, C], f16, tag="imgg")
        for cs in range(4):
            stage32 = imgstage.tile([128, HW], f32, tag="s32")
            nc.sync.dma_start(stage32[:, :],
                              img_in.ap()[cs * 128:(cs + 1) * 128, :])
            stage16 = imgstage.tile([128, HW], f16, tag="s16")
            nc.vector.tensor_copy(stage16[:, :], stage32[:, :])
            nc.sync.dma_start_transpose(
                img_g[:, :, cs * 128:(cs + 1) * 128], stage16[:, :])
        nc.sync.dma_start(
            img_nhwc[:, :].rearrange("(j p) c -> p j c", p=128),
            img_g[:, :, :])

        # ------------------------------------------------ roi prep
        # roisT[0, c*R_PAD + r] = rois[r, c]
        roisT = prep_pool.tile([1, 4 * R_PAD], f32, tag="roisT")
        nc.sync.dma_start(roisT[:, :].rearrange("o (c r) -> o c r", c=4),
                          rois_in.ap()[:, :].rearrange("r c -> c r"))

        # bc row: [y1n, x1n, dyn, dxn] each R_PAD wide
        bc = prep_pool.tile([64, 4 * R_PAD], f32, tag="bc")
        inv_h = 1.0 / (IH - 1.0)
        inv_w = 1.0 / (IW - 1.0)
        r0 = bc[0:1, :]
        nc.vector.tensor_scalar_mul(r0[:, 0:R_PAD], roisT[:, 0:R_PAD], inv_h)
        nc.vector.tensor_scalar_mul(r0[:, R_PAD:2 * R_PAD],
                                    roisT[:, R_PAD:2 * R_PAD], inv_w)
        tmp = prep_pool.tile([1, 2 * R_PAD], f32, tag="tmp2")
        nc.vector.tensor_scalar_mul(tmp[:, 0:R_PAD],
                                    roisT[:, 2 * R_PAD:3 * R_PAD], inv_h)
        nc.vector.tensor_scalar_mul(tmp[:, R_PAD:2 * R_PAD],
                                    roisT[:, 3 * R_PAD:4 * R_PAD], inv_w)
        nc.vector.tensor_sub(r0[:, 2 * R_PAD:3 * R_PAD], tmp[:, 0:R_PAD],
                             r0[:, 0:R_PAD])
        nc.vector.tensor_sub(r0[:, 3 * R_PAD:4 * R_PAD],
                             tmp[:, R_PAD:2 * R_PAD], r0[:, R_PAD:2 * R_PAD])
        # broadcast row 0 -> 64 partitions (need 49) by doubling
        for k in range(6):
            lo, hi = 1 << k, min(2 << k, 64)
            nc.sync.dma_start(bc[lo:hi, :], bc[0:lo, :][0:hi - lo, :])

        y1n = bc[0:PTS, 0:R_PAD]
        x1n = bc[0:PTS, R_PAD:2 * R_PAD]
        dyn = bc[0:PTS, 2 * R_PAD:3 * R_PAD]
        dxn = bc[0:PTS, 3 * R_PAD:4 * R_PAD]

        def ptile(nm, dt_=None):
            return prep_pool.tile([PTS, R_PAD], dt_ or f32, tag=nm, name=nm)

        def prep_axis(ax, gcol, lo_n, d_n, hdim):
            """returns (c0f, cbf, lc, mc) tiles [49, R_PAD] f32"""
            inn = ptile(f"inn{ax}")
            nc.vector.scalar_tensor_tensor(inn[:, :], d_n, gcol[:, :], lo_n,
                                           Alu.mult, Alu.add)
            nc.vector.tensor_scalar_mul(inn[:, :], inn[:, :], hdim - 1.0)
            cc = ptile(f"cc{ax}")
            nc.vector.tensor_scalar(cc[:, :], inn[:, :], 0.0, hdim - 1.0,
                                    Alu.max, Alu.min)
            # exact floor for 0<=x<2^22: t=(x+2^23)-2^23 is round-nearest;
            # subtract 1 where t > x
            rnd = ptile(f"rnd{ax}")
            nc.vector.tensor_scalar(rnd[:, :], cc[:, :], 8388608.0, 8388608.0,
                                    Alu.add, Alu.subtract)
            gt = ptile(f"gt{ax}")
            nc.vector.tensor_tensor(gt[:, :], rnd[:, :], cc[:, :], Alu.is_gt)
            c0f = ptile(f"c0f{ax}")
            nc.vector.tensor_sub(c0f[:, :], rnd[:, :], gt[:, :])
            cbf = ptile(f"cbf{ax}")
            nc.vector.tensor_scalar(cbf[:, :], c0f[:, :], 1.0, hdim - 1.0,
                                    Alu.add, Alu.min)
            lc = ptile(f"lc{ax}")
            nc.vector.tensor_sub(lc[:, :], inn[:, :], c0f[:, :])
            m1 = ptile(f"m1{ax}")
            nc.vector.tensor_scalar(m1[:, :], inn[:, :], 0.0, None, Alu.is_ge)
            m2 = ptile(f"m2{ax}")
            nc.vector.tensor_scalar(m2[:, :], inn[:, :], hdim - 1.0, None,
                                    Alu.is_le)
            mc = ptile(f"mc{ax}")
            nc.vector.tensor_mul(mc[:, :], m1[:, :], m2[:, :])
            return inn, c0f, cbf, lc, mc

        _, y0f, ybf, ly, my = prep_axis("y", gy_col, y1n, dyn, float(H))
        _, x0f, xbf, lx, mx = prep_axis("x", gx_col, x1n, dxn, float(W))

        def ab(ax_, lc, mc):
            a = ptile(f"a{ax_}")
            nc.vector.tensor_scalar(a[:, :], lc[:, :], -1.0, 1.0, Alu.mult,
                                    Alu.add)
            nc.vector.tensor_mul(a[:, :], a[:, :], mc[:, :])
            b = ptile(f"b{ax_}")
            nc.vector.tensor_mul(b[:, :], lc[:, :], mc[:, :])
            return a, b

        ay, by = ab("y", ly, my)
        ax, bx = ab("x", lx, mx)

        # per-corner weights (fp16) and indices (int16)
        corners = []  # (w16 tile, idx16 tile)
        for kc, (wy, wx_, yf, xf) in enumerate(
                ((ay, ax, y0f, x0f), (ay, bx, y0f, xbf),
                 (by, ax, ybf, x0f), (by, bx, ybf, xbf))):
            w16 = ptile(f"w16_{kc}", f16)
            nc.vector.tensor_mul(w16[:, :], wy[:, :], wx_[:, :])
            idxf = ptile(f"idxf{kc}")
            nc.vector.scalar_tensor_tensor(idxf[:, :], yf[:, :], float(W),
                                           xf[:, :], Alu.mult, Alu.add)
            idx16 = ptile(f"idx16_{kc}", i16)
            nc.vector.tensor_copy(idx16[:, :], idxf[:, :])
            corners.append((w16, idx16))

        # ------------------------------------------------ idx wrap + W flatten
        # gather order within batch b: j = rl*64 + pt  (rl<16, pt<49 valid)
        # wrapped: partition p = pt%16 (q=pt//16<4), slot s = rl*4 + q
        # idxw[k] free layout: [b(10), s(64)]
        idxw = const_pool.tile([128, 4, NBATCH, SPB], i16, tag="idxw")
        nc.gpsimd.memset(idxw[:, :, :, :], 0)
        # wflat: partition k holds corner k's flat row [b(10), rl(16), pt-slot(64)]
        wdram = dram_pool.tile([4, NBATCH * NIDX], f16, name="wdram")
        for k, (w16, idx16) in enumerate(corners):
            # idx wrap: dst[p, k, b, rl*4+q] = idx16[q*16+p, b*16+rl]
            for q in range(4):
                npq = min(16, PTS - q * 16)  # 16,16,16,1
                src = idx16[q * 16:q * 16 + npq, :].rearrange(
                    "p (b r) -> p b r", b=NBATCH)
                dst = idxw[0:npq, k, :, :].rearrange(
                    "p b (r q) -> p b r q", q=4)[:, :, :, q]
                nc.sync.dma_start(dst, src)
            # w flatten: wflat[k, b*1024 + rl*64 + pt] = w16[pt, b*16+rl]
            # dst iterated (s, b, r) to match src element order (p, b, r)
            dstw = wdram[k:k + 1, :].rearrange(
                "o (b r s) -> o s b r", b=NBATCH, r=RB)[:, 0:PTS, :, :]
            nc.sync.dma_start(dstw, w16[:, :].rearrange(
                "p (b r) -> p b r", b=NBATCH))
        for k in range(3):
            lo, hi = 16 << k, 32 << k
            nc.sync.dma_start(idxw[lo:hi, :, :, :], idxw[0:hi - lo, :, :, :])

        # ------------------------------------------------ main loop
        for b in range(NBATCH):
            # rois beyond R_CORE are host-side padding; skip fully-pad batches
            nv = RB if (b + 1) * RB <= R_CORE else R_CORE - b * RB
            if nv <= 0:
                continue
            ob = o_pool.tile([128, 4, RB, PTS], f16, tag="O")
            for k in range(4):
                tk = gather_pool.tile([128, 4, NIDX], f16, tag="T")
                nc.gpsimd.dma_gather(
                    tk[:, :, :], img_nhwc[:, :], idxw[:, k, b, :],
                    NIDX, NIDX, C,
                    transpose=True,
                )
                wrow = wrow_pool.tile([1, NIDX], f16, tag="wr")
                nc.sync.dma_start(wrow[:, :],
                                  wdram[k:k + 1, b * NIDX:(b + 1) * NIDX])
                wk = w_pool.tile([128, NIDX], f16, tag="W")
                ps = psum_pool.tile([128, NIDX], f32, tag="ps")
                nc.tensor.matmul(ps[:, :], ones16[:, :], wrow[:, :],
                                 start=True, stop=True)
                nc.scalar.copy(wk[:, :], ps[:, :])
                # valid-slot views [128, 4, RB, PTS]
                tv = tk[:, :, :].rearrange("p e (r s) -> p e r s",
                                           r=RB)[:, :, :, 0:PTS]
                wv = wk[:, :].rearrange("p (r s) -> p r s",
                                        r=RB)[:, :, 0:PTS]
                wv4 = wv  # broadcast over e by explicit per-e ops
                if k == 0:
                    for e in range(4):
                        nc.vector.tensor_mul(ob[:, e, :, :], tv[:, e, :, :],
                                             wv4)
                else:
                    for e in range(4):
                        nc.vector.tensor_mul(tv[:, e, :, :], tv[:, e, :, :],
                                             wv4)
                    nc.vector.tensor_add(ob[:, :, :, :], ob[:, :, :, :], tv)

            # output write with cast fp16 -> f32
            # dst out[b*16+rl, e*128+p, py, px]; 3-dim AP limit -> per-e DMA
            dste = out_t.ap()[b * RB:b * RB + nv, :, :, :].rearrange(
                "r (e p) py px -> p e r (py px)", e=4)
            for e in range(4):
                nc.gpsimd.dma_start(dste[:, e, :, :], ob[:, e, 0:nv, :])


def _get_program():
    if "nc" not in _prog_cache:
        _prog_cache["nc"] = _build_program()
    return _prog_cache["nc"]


def _make_consts():
    consts = np.zeros((1, 256), dtype=np.float32)
    g = (np.arange(POOL, dtype=np.float32) / np.float32(POOL - 1.0)).astype(
        np.float32)
    gy = np.repeat(g, POOL)   # g[pt//7]
    gx = np.tile(g, POOL)     # g[pt%7]
    consts[0, 0:PTS] = gy
    consts[0, PTS:2 * PTS] = gx
    consts[0, 98:226] = 1.0
    return consts


def kernel(img: np.ndarray, rois: np.ndarray,
           input_image: np.ndarray) -> np.ndarray:
    from concourse.bass_utils import run_bass_kernel_spmd

    nc = _get_program()
    consts = _make_consts()
    in_maps = []
    for c in range(N_CORES):
        n, half = c // 2, c % 2
        rpad = np.zeros((R_PAD, 4), dtype=np.float32)
        rpad[:R_CORE] = rois[n, half * R_CORE:(half + 1) * R_CORE]
        in_maps.append({
            "img": np.ascontiguousarray(
                img[n].reshape(C, HW).astype(np.float32)),
            "rois": rpad,
            "consts": consts,
        })
    res = run_bass_kernel_spmd(nc, in_maps, core_ids=list(range(N_CORES)))
    out = np.empty((N, B, C, POOL, POOL), dtype=np.float32)
    for c in range(N_CORES):
        n, half = c // 2, c % 2
        out[n, half * R_CORE:(half + 1) * R_CORE] = res.results[c]["out"]
    return out



# revision 4
# speedup vs baseline: 1.2764x; 1.2764x over previous
"""CropAndResize (tf.image.crop_and_resize semantics, bilinear, extrap=0)
Trainium2 Bass kernel, data-parallel over 8 NeuronCores.

Full inputs:  img (4,512,64,64) f32, rois (4,300,4) f32, input_image (4,3,1024,1024) f32
Full output:  (4,300,512,7,7) f32

Sharding: core c handles image n = c//2 and that image's roi slice
[ (c%2)*150 : (c%2)*150+150 ] (padded to 160 = 10 batches of 16).

Per-core device program (fp16 compute, f32 in/out):
  1. img NCHW f32 -> SBUF -> cast fp16 -> xbar DMA-transpose into the
     gather layout img_g[p, j, c] = row(hw=j*128+p), 1024B/row.
  2. RoI prep on DVE in [49pt, 160roi] layout: sample coords, clipped
     floors, lerp weights folded with the validity mask, gather indices.
  3. Index wrap into the dma_gather int16 [16-wrapped, replicated] layout
     and weight flatten to a per-corner row, via small DMAs.
  4. Per batch (16 rois): 4 SBUF-source transpose-mode dma_gathers
     (channels land on partitions), PE ones-matmul broadcast of weights,
     DVE lerp: out = sum_k T_k * W_k, write f32 out with strided DMA.
"""

import os
import sys

import numpy as np

_RL_REPO_CANDIDATES = ["/opt/trn_rl_repo", "/root/.axon_site/_ro/trn_rl_repo"]
for _p in _RL_REPO_CANDIDATES:
    if os.path.isdir(_p) and _p not in sys.path:
        sys.path.insert(0, _p)

import ml_dtypes  # noqa: E402

# ---------------------------------------------------------------- constants
N_CORES = 8
N, C, H, W = 4, 512, 64, 64
B = 300
POOL = 7
PTS = POOL * POOL  # 49
IH, IW = 1024.0, 1024.0
R_CORE = B // 2          # 150 real rois per core
R_PAD = 160              # padded roi count per core
RB = 8                   # rois per batch
NBATCH = R_PAD // RB     # 10
SLOT = 64                # per-roi slot stride in the gather index space
NIDX = RB * SLOT         # 1024 gather indices per batch (mult of 128)
SPB = NIDX // 16         # 64 wrapped-idx free slots per batch
HW = H * W               # 4096

_prog_cache = {}


def _build_program():
    import concourse.bass as bass
    import concourse.bacc as bacc
    import concourse.mybir as mybir
    import concourse.tile as tile

    f32 = mybir.dt.float32
    f16 = mybir.dt.float16
    i16 = mybir.dt.int16
    Alu = mybir.AluOpType

    nc = bacc.Bacc("TRN2", target_bir_lowering=False, debug=False,
                   num_devices=N_CORES)

    img_in = nc.dram_tensor("img", (C, HW), f32, kind="ExternalInput")
    rois_in = nc.dram_tensor("rois", (R_PAD, 4), f32, kind="ExternalInput")
    consts_in = nc.dram_tensor("consts", (1, 256), f32, kind="ExternalInput")
    out_t = nc.dram_tensor("out", (R_CORE, C, POOL, POOL), f32,
                           kind="ExternalOutput")

    with tile.TileContext(nc) as tc:
        _body(tc, nc, bass, mybir, tile, img_in, rois_in, consts_in, out_t,
              f32, f16, i16, Alu)

    nc.compile()
    return nc


def _body(tc, nc, bass, mybir, tile, img_in, rois_in, consts_in, out_t,
          f32, f16, i16, Alu):
    from contextlib import ExitStack
    ctx = ExitStack()
    with ctx:
        import os as _os
        GB = int(_os.environ.get("K_GBUFS", "3"))
        WB = int(_os.environ.get("K_WBUFS", "3"))
        OB = int(_os.environ.get("K_OBUFS", "2"))
        PB = int(_os.environ.get("K_PBUFS", "4"))
        const_pool = ctx.enter_context(tc.tile_pool(name="const", bufs=1))
        prep_pool = ctx.enter_context(tc.tile_pool(name="prep", bufs=1))
        imgstage = ctx.enter_context(tc.tile_pool(name="imgstage", bufs=2))
        gather_pool = ctx.enter_context(tc.tile_pool(name="gather", bufs=GB))
        w_pool = ctx.enter_context(tc.tile_pool(name="wts", bufs=WB))
        wrow_pool = ctx.enter_context(tc.tile_pool(name="wrow", bufs=WB))
        o_pool = ctx.enter_context(tc.tile_pool(name="outs", bufs=OB))
        dram_pool = ctx.enter_context(
            tc.tile_pool(name="dram", bufs=1, space="DRAM"))
        psum_pool = ctx.enter_context(
            tc.tile_pool(name="psum", bufs=PB, space="PSUM"))

        # ------------------------------------------------ constants
        # consts row: [0:49]=g_y per pt, [49:98]=g_x per pt, [98:226]=ones
        gy_col = const_pool.tile([PTS, 1], f32, tag="gy")
        gx_col = const_pool.tile([PTS, 1], f32, tag="gx")
        # strided loads: partition p <- consts[0, p] / consts[0, 49+p]
        nc.sync.dma_start(gy_col[:, :], consts_in.ap()[0:1, 0:PTS].rearrange(
            "a p -> p a"))
        nc.sync.dma_start(gx_col[:, :], consts_in.ap()[0:1, PTS:2 * PTS]
                          .rearrange("a p -> p a"))
        ones_f32 = const_pool.tile([1, 128], f32, tag="ones32")
        nc.sync.dma_start(ones_f32[:, :], consts_in.ap()[0:1, 98:226])
        ones16 = const_pool.tile([1, 128], f16, tag="ones16")
        nc.vector.tensor_copy(ones16[:, :], ones_f32[:, :])

        # ------------------------------------------------ image prep
        # img_g[p, j, c] = img row hw=j*128+p, fp16, gather source in SBUF
        # (sbuf_tokens_per_rank=128: idx -> rank=idx//128 (byte off rank*1024),
        #  tok=idx%128 (partition) -- matches hw = j*128 + p).
        img_g = const_pool.tile([128, 32, C], f16, tag="imgg")
        for cs in range(4):
            stage32 = imgstage.tile([128, HW], f32, tag="s32")
            nc.sync.dma_start(stage32[:, :],
                              img_in.ap()[cs * 128:(cs + 1) * 128, :])
            stage16 = imgstage.tile([128, HW], f16, tag="s16")
            nc.vector.tensor_copy(stage16[:, :], stage32[:, :])
            nc.sync.dma_start_transpose(
                img_g[:, :, cs * 128:(cs + 1) * 128], stage16[:, :])

        # ------------------------------------------------ roi prep
        # roisT[0, c*R_PAD + r] = rois[r, c]
        roisT = prep_pool.tile([1, 4 * R_PAD], f32, tag="roisT")
        nc.sync.dma_start(roisT[:, :].rearrange("o (c r) -> o c r", c=4),
                          rois_in.ap()[:, :].rearrange("r c -> c r"))

        # bc row: [y1n, x1n, dyn, dxn] each R_PAD wide
        bc = prep_pool.tile([64, 4 * R_PAD], f32, tag="bc")
        inv_h = 1.0 / (IH - 1.0)
        inv_w = 1.0 / (IW - 1.0)
        r0 = bc[0:1, :]
        nc.vector.tensor_scalar_mul(r0[:, 0:R_PAD], roisT[:, 0:R_PAD], inv_h)
        nc.vector.tensor_scalar_mul(r0[:, R_PAD:2 * R_PAD],
                                    roisT[:, R_PAD:2 * R_PAD], inv_w)
        tmp = prep_pool.tile([1, 2 * R_PAD], f32, tag="tmp2")
        nc.vector.tensor_scalar_mul(tmp[:, 0:R_PAD],
                                    roisT[:, 2 * R_PAD:3 * R_PAD], inv_h)
        nc.vector.tensor_scalar_mul(tmp[:, R_PAD:2 * R_PAD],
                                    roisT[:, 3 * R_PAD:4 * R_PAD], inv_w)
        nc.vector.tensor_sub(r0[:, 2 * R_PAD:3 * R_PAD], tmp[:, 0:R_PAD],
                             r0[:, 0:R_PAD])
        nc.vector.tensor_sub(r0[:, 3 * R_PAD:4 * R_PAD],
                             tmp[:, R_PAD:2 * R_PAD], r0[:, R_PAD:2 * R_PAD])
        # broadcast row 0 -> 64 partitions (need 49) by doubling
        for k in range(6):
            lo, hi = 1 << k, min(2 << k, 64)
            nc.sync.dma_start(bc[lo:hi, :], bc[0:lo, :][0:hi - lo, :])

        y1n = bc[0:PTS, 0:R_PAD]
        x1n = bc[0:PTS, R_PAD:2 * R_PAD]
        dyn = bc[0:PTS, 2 * R_PAD:3 * R_PAD]
        dxn = bc[0:PTS, 3 * R_PAD:4 * R_PAD]

        def ptile(nm, dt_=None):
            return prep_pool.tile([PTS, R_PAD], dt_ or f32, tag=nm, name=nm)

        def prep_axis(ax, gcol, lo_n, d_n, hdim):
            """returns (c0f, cbf, lc, mc) tiles [49, R_PAD] f32"""
            inn = ptile(f"inn{ax}")
            nc.vector.scalar_tensor_tensor(inn[:, :], d_n, gcol[:, :], lo_n,
                                           Alu.mult, Alu.add)
            nc.vector.tensor_scalar_mul(inn[:, :], inn[:, :], hdim - 1.0)
            cc = ptile(f"cc{ax}")
            nc.vector.tensor_scalar(cc[:, :], inn[:, :], 0.0, hdim - 1.0,
                                    Alu.max, Alu.min)
            # exact floor for 0<=x<2^22: t=(x+2^23)-2^23 is round-nearest;
            # subtract 1 where t > x
            rnd = ptile(f"rnd{ax}")
            nc.vector.tensor_scalar(rnd[:, :], cc[:, :], 8388608.0, 8388608.0,
                                    Alu.add, Alu.subtract)
            gt = ptile(f"gt{ax}")
            nc.vector.tensor_tensor(gt[:, :], rnd[:, :], cc[:, :], Alu.is_gt)
            c0f = ptile(f"c0f{ax}")
            nc.vector.tensor_sub(c0f[:, :], rnd[:, :], gt[:, :])
            cbf = ptile(f"cbf{ax}")
            nc.vector.tensor_scalar(cbf[:, :], c0f[:, :], 1.0, hdim - 1.0,
                                    Alu.add, Alu.min)
            lc = ptile(f"lc{ax}")
            nc.vector.tensor_sub(lc[:, :], inn[:, :], c0f[:, :])
            m1 = ptile(f"m1{ax}")
            nc.vector.tensor_scalar(m1[:, :], inn[:, :], 0.0, None, Alu.is_ge)
            m2 = ptile(f"m2{ax}")
            nc.vector.tensor_scalar(m2[:, :], inn[:, :], hdim - 1.0, None,
                                    Alu.is_le)
            mc = ptile(f"mc{ax}")
            nc.vector.tensor_mul(mc[:, :], m1[:, :], m2[:, :])
            return inn, c0f, cbf, lc, mc

        _, y0f, ybf, ly, my = prep_axis("y", gy_col, y1n, dyn, float(H))
        _, x0f, xbf, lx, mx = prep_axis("x", gx_col, x1n, dxn, float(W))

        def ab(ax_, lc, mc):
            a = ptile(f"a{ax_}")
            nc.vector.tensor_scalar(a[:, :], lc[:, :], -1.0, 1.0, Alu.mult,
                                    Alu.add)
            nc.vector.tensor_mul(a[:, :], a[:, :], mc[:, :])
            b = ptile(f"b{ax_}")
            nc.vector.tensor_mul(b[:, :], lc[:, :], mc[:, :])
            return a, b

        ay, by = ab("y", ly, my)
        ax, bx = ab("x", lx, mx)

        # per-corner weights (fp16) and indices (int16)
        corners = []  # (w16 tile, idx16 tile)
        for kc, (wy, wx_, yf, xf) in enumerate(
                ((ay, ax, y0f, x0f), (ay, bx, y0f, xbf),
                 (by, ax, ybf, x0f), (by, bx, ybf, xbf))):
            w16 = ptile(f"w16_{kc}", f16)
            nc.vector.tensor_mul(w16[:, :], wy[:, :], wx_[:, :])
            idxf = ptile(f"idxf{kc}")
            nc.vector.scalar_tensor_tensor(idxf[:, :], yf[:, :], float(W),
                                           xf[:, :], Alu.mult, Alu.add)
            idx16 = ptile(f"idx16_{kc}", i16)
            nc.vector.tensor_copy(idx16[:, :], idxf[:, :])
            corners.append((w16, idx16))

        # ------------------------------------------------ idx wrap + W flatten
        # gather order within batch b: j = rl*64 + pt  (rl<16, pt<49 valid)
        # wrapped: partition p = pt%16 (q=pt//16<4), slot s = rl*4 + q
        # idxw[k] free layout: [b(10), s(64)]
        idxw = const_pool.tile([128, 4, NBATCH, SPB], i16, tag="idxw")
        nc.gpsimd.memset(idxw[:, :, :, :], 0)
        # wflat: partition k holds corner k's flat row [b(10), rl(16), pt-slot(64)]
        wdram = dram_pool.tile([4, NBATCH * NIDX], f16, name="wdram")
        for k, (w16, idx16) in enumerate(corners):
            # idx wrap: dst[p, k, b, rl*4+q] = idx16[q*16+p, b*16+rl]
            for q in range(4):
                npq = min(16, PTS - q * 16)  # 16,16,16,1
                src = idx16[q * 16:q * 16 + npq, :].rearrange(
                    "p (b r) -> p b r", b=NBATCH)
                dst = idxw[0:npq, k, :, :].rearrange(
                    "p b (r q) -> p b r q", q=4)[:, :, :, q]
                nc.sync.dma_start(dst, src)
            # w flatten: wflat[k, b*1024 + rl*64 + pt] = w16[pt, b*16+rl]
            # dst iterated (s, b, r) to match src element order (p, b, r)
            dstw = wdram[k:k + 1, :].rearrange(
                "o (b r s) -> o s b r", b=NBATCH, r=RB)[:, 0:PTS, :, :]
            nc.sync.dma_start(dstw, w16[:, :].rearrange(
                "p (b r) -> p b r", b=NBATCH))
        for k in range(3):
            lo, hi = 16 << k, 32 << k
            nc.sync.dma_start(idxw[lo:hi, :, :, :], idxw[0:hi - lo, :, :, :])

        # ------------------------------------------------ main loop
        for b in range(NBATCH):
            # rois beyond R_CORE are host-side padding; skip fully-pad batches
            nv = RB if (b + 1) * RB <= R_CORE else R_CORE - b * RB
            if nv <= 0:
                continue
            ob = o_pool.tile([128, 4, RB, PTS], f16, tag="O")
            for k in range(4):
                tk = gather_pool.tile([128, 4, NIDX], f16, tag="T")
                nc.gpsimd.dma_gather(
                    tk[:, :, :], img_g[:, :, :], idxw[:, k, b, :],
                    NIDX, NIDX, C,
                    transpose=True,
                    sbuf_tokens_per_rank=128,
                    sbuf_free_dim_per_rank=C * 2,
                )
                wrow = wrow_pool.tile([1, NIDX], f16, tag="wr")
                nc.sync.dma_start(wrow[:, :],
                                  wdram[k:k + 1, b * NIDX:(b + 1) * NIDX])
                wk = w_pool.tile([128, NIDX], f16, tag="W")
                ps = psum_pool.tile([128, NIDX], f32, tag="ps")
                nc.tensor.matmul(ps[:, :], ones16[:, :], wrow[:, :],
                                 start=True, stop=True)
                nc.scalar.copy(wk[:, :], ps[:, :])
                # valid-slot views [128, 4, RB, PTS]
                tv = tk[:, :, :].rearrange("p e (r s) -> p e r s",
                                           r=RB)[:, :, :, 0:PTS]
                wv = wk[:, :].rearrange("p (r s) -> p r s",
                                        r=RB)[:, :, 0:PTS]
                wv4 = wv  # broadcast over e by explicit per-e ops
                if k == 0:
                    for e in range(4):
                        nc.vector.tensor_mul(ob[:, e, :, :], tv[:, e, :, :],
                                             wv4)
                else:
                    for e in range(4):
                        nc.vector.tensor_mul(tv[:, e, :, :], tv[:, e, :, :],
                                             wv4)
                    nc.vector.tensor_add(ob[:, :, :, :], ob[:, :, :, :], tv)

            # cast fp16 -> f32 on Activation, then HWDGE write (no SWDGE gen)
            ob32 = o_pool.tile([128, 4, RB, PTS], f32, tag="O32")
            nc.scalar.copy(ob32[:, :, :, :], ob[:, :, :, :])
            dste = out_t.ap()[b * RB:b * RB + nv, :, :, :].rearrange(
                "r (e p) py px -> p e r (py px)", e=4)
            for e in range(4):
                nc.sync.dma_start(dste[:, e, :, :], ob32[:, e, 0:nv, :])


def _get_program():
    if "nc" not in _prog_cache:
        _prog_cache["nc"] = _build_program()
    return _prog_cache["nc"]


def _make_consts():
    consts = np.zeros((1, 256), dtype=np.float32)
    g = (np.arange(POOL, dtype=np.float32) / np.float32(POOL - 1.0)).astype(
        np.float32)
    gy = np.repeat(g, POOL)   # g[pt//7]
    gx = np.tile(g, POOL)     # g[pt%7]
    consts[0, 0:PTS] = gy
    consts[0, PTS:2 * PTS] = gx
    consts[0, 98:226] = 1.0
    return consts


def kernel(img: np.ndarray, rois: np.ndarray,
           input_image: np.ndarray) -> np.ndarray:
    from concourse.bass_utils import run_bass_kernel_spmd

    nc = _get_program()
    consts = _make_consts()
    in_maps = []
    for c in range(N_CORES):
        n, half = c // 2, c % 2
        rpad = np.zeros((R_PAD, 4), dtype=np.float32)
        rpad[:R_CORE] = rois[n, half * R_CORE:(half + 1) * R_CORE]
        in_maps.append({
            "img": np.ascontiguousarray(
                img[n].reshape(C, HW).astype(np.float32)),
            "rois": rpad,
            "consts": consts,
        })
    res = run_bass_kernel_spmd(nc, in_maps, core_ids=list(range(N_CORES)))
    out = np.empty((N, B, C, POOL, POOL), dtype=np.float32)
    for c in range(N_CORES):
        n, half = c // 2, c % 2
        out[n, half * R_CORE:(half + 1) * R_CORE] = res.results[c]["out"]
    return out



# revision 9
# speedup vs baseline: 1.4549x; 1.1398x over previous
"""CropAndResize (tf.image.crop_and_resize semantics, bilinear, extrap=0)
Trainium2 Bass kernel, data-parallel over 8 NeuronCores.

Full inputs:  img (4,512,64,64) f32, rois (4,300,4) f32, input_image (4,3,1024,1024) f32
Full output:  (4,300,512,7,7) f32

Sharding: core c handles image n = c//2 and that image's roi slice
[ (c%2)*150 : (c%2)*150+150 ] (padded to 160 = 10 batches of 16).

Per-core device program (fp16 compute, f32 in/out):
  1. img NCHW f32 -> SBUF -> cast fp16 -> xbar DMA-transpose into the
     gather layout img_g[p, j, c] = row(hw=j*128+p), 1024B/row.
  2. RoI prep on DVE in [49pt, 160roi] layout: sample coords, clipped
     floors, lerp weights folded with the validity mask, gather indices.
  3. Index wrap into the dma_gather int16 [16-wrapped, replicated] layout
     and weight flatten to a per-corner row, via small DMAs.
  4. Per batch (16 rois): 4 SBUF-source transpose-mode dma_gathers
     (channels land on partitions), PE ones-matmul broadcast of weights,
     DVE lerp: out = sum_k T_k * W_k, write f32 out with strided DMA.
"""

import os
import sys

import numpy as np

_RL_REPO_CANDIDATES = ["/opt/trn_rl_repo", "/root/.axon_site/_ro/trn_rl_repo"]
for _p in _RL_REPO_CANDIDATES:
    if os.path.isdir(_p) and _p not in sys.path:
        sys.path.insert(0, _p)

import ml_dtypes  # noqa: E402

# ---------------------------------------------------------------- constants
N_CORES = 8
N, C, H, W = 4, 512, 64, 64
B = 300
POOL = 7
PTS = POOL * POOL  # 49
IH, IW = 1024.0, 1024.0
R_CORE = B // 2          # 150 real rois per core
R_PAD = 160              # padded roi count per core
RB = 8                   # rois per batch
NBATCH = R_PAD // RB     # 10
SLOT = 64                # per-roi slot stride in the gather index space
NIDX = RB * SLOT         # 1024 gather indices per batch (mult of 128)
SPB = NIDX // 16         # 64 wrapped-idx free slots per batch
HW = H * W               # 4096

_prog_cache = {}


def _build_program():
    import concourse.bass as bass
    import concourse.bacc as bacc
    import concourse.mybir as mybir
    import concourse.tile as tile

    f32 = mybir.dt.float32
    f16 = mybir.dt.float16
    i16 = mybir.dt.int16
    Alu = mybir.AluOpType

    nc = bacc.Bacc("TRN2", target_bir_lowering=False, debug=False,
                   num_devices=N_CORES)

    img_in = nc.dram_tensor("img", (HW, C), f16, kind="ExternalInput")
    rois_in = nc.dram_tensor("rois", (R_PAD, 4), f32, kind="ExternalInput")
    consts_in = nc.dram_tensor("consts", (1, 256), f32, kind="ExternalInput")
    out_t = nc.dram_tensor("out", (R_CORE, C, POOL, POOL), f32,
                           kind="ExternalOutput")

    with tile.TileContext(nc) as tc:
        _body(tc, nc, bass, mybir, tile, img_in, rois_in, consts_in, out_t,
              f32, f16, i16, Alu)

    nc.compile()
    return nc


def _body(tc, nc, bass, mybir, tile, img_in, rois_in, consts_in, out_t,
          f32, f16, i16, Alu):
    from contextlib import ExitStack
    ctx = ExitStack()
    with ctx:
        import os as _os
        GB = int(_os.environ.get("K_GBUFS", "3"))
        WB = int(_os.environ.get("K_WBUFS", "3"))
        OB = int(_os.environ.get("K_OBUFS", "2"))
        PB = int(_os.environ.get("K_PBUFS", "4"))
        const_pool = ctx.enter_context(tc.tile_pool(name="const", bufs=1))
        prep_pool = ctx.enter_context(tc.tile_pool(name="prep", bufs=1))
        gather_pool = ctx.enter_context(tc.tile_pool(name="gather", bufs=GB))
        w_pool = ctx.enter_context(tc.tile_pool(name="wts", bufs=WB))
        wrow_pool = ctx.enter_context(tc.tile_pool(name="wrow", bufs=WB))
        o_pool = ctx.enter_context(tc.tile_pool(name="outs", bufs=OB))
        dram_pool = ctx.enter_context(
            tc.tile_pool(name="dram", bufs=1, space="DRAM"))
        psum_pool = ctx.enter_context(
            tc.tile_pool(name="psum", bufs=PB, space="PSUM"))

        # ------------------------------------------------ constants
        # consts row: [0:49]=g_y per pt, [49:98]=g_x per pt, [98:226]=ones
        gy_col = const_pool.tile([PTS, 1], f32, tag="gy")
        gx_col = const_pool.tile([PTS, 1], f32, tag="gx")
        # strided loads: partition p <- consts[0, p] / consts[0, 49+p]
        nc.sync.dma_start(gy_col[:, :], consts_in.ap()[0:1, 0:PTS].rearrange(
            "a p -> p a"))
        nc.sync.dma_start(gx_col[:, :], consts_in.ap()[0:1, PTS:2 * PTS]
                          .rearrange("a p -> p a"))
        ones_f32 = const_pool.tile([1, 128], f32, tag="ones32")
        nc.sync.dma_start(ones_f32[:, :], consts_in.ap()[0:1, 98:226])
        ones16 = const_pool.tile([1, 128], f16, tag="ones16")
        nc.vector.tensor_copy(ones16[:, :], ones_f32[:, :])

        # ------------------------------------------------ image prep
        # img_g[p, j, c] = img row hw=j*128+p, fp16, gather source in SBUF
        # (sbuf_tokens_per_rank=128: idx -> rank=idx//128 (byte off rank*1024),
        #  tok=idx%128 (partition) -- matches hw = j*128 + p).
        # img input is host-pretransposed NHWC fp16 [HW, C]: direct load.
        img_g = const_pool.tile([128, 32, C], f16, tag="imgg")
        nc.sync.dma_start(img_g[:, :, :],
                          img_in.ap()[:, :].rearrange("(g p) c -> p g c",
                                                      p=128))

        # ------------------------------------------------ roi prep
        # roisT[0, c*R_PAD + r] = rois[r, c]
        roisT = prep_pool.tile([1, 4 * R_PAD], f32, tag="roisT")
        nc.sync.dma_start(roisT[:, :].rearrange("o (c r) -> o c r", c=4),
                          rois_in.ap()[:, :].rearrange("r c -> c r"))

        # bc row: [y1n, x1n, dyn, dxn] each R_PAD wide
        bc = prep_pool.tile([64, 4 * R_PAD], f32, tag="bc")
        inv_h = 1.0 / (IH - 1.0)
        inv_w = 1.0 / (IW - 1.0)
        r0 = bc[0:1, :]
        nc.vector.tensor_scalar_mul(r0[:, 0:R_PAD], roisT[:, 0:R_PAD], inv_h)
        nc.vector.tensor_scalar_mul(r0[:, R_PAD:2 * R_PAD],
                                    roisT[:, R_PAD:2 * R_PAD], inv_w)
        tmp = prep_pool.tile([1, 2 * R_PAD], f32, tag="tmp2")
        nc.vector.tensor_scalar_mul(tmp[:, 0:R_PAD],
                                    roisT[:, 2 * R_PAD:3 * R_PAD], inv_h)
        nc.vector.tensor_scalar_mul(tmp[:, R_PAD:2 * R_PAD],
                                    roisT[:, 3 * R_PAD:4 * R_PAD], inv_w)
        nc.vector.tensor_sub(r0[:, 2 * R_PAD:3 * R_PAD], tmp[:, 0:R_PAD],
                             r0[:, 0:R_PAD])
        nc.vector.tensor_sub(r0[:, 3 * R_PAD:4 * R_PAD],
                             tmp[:, R_PAD:2 * R_PAD], r0[:, R_PAD:2 * R_PAD])
        # broadcast row 0 -> 64 partitions (need 49) by doubling
        for k in range(6):
            lo, hi = 1 << k, min(2 << k, 64)
            nc.sync.dma_start(bc[lo:hi, :], bc[0:lo, :][0:hi - lo, :])

        y1n = bc[0:PTS, 0:R_PAD]
        x1n = bc[0:PTS, R_PAD:2 * R_PAD]
        dyn = bc[0:PTS, 2 * R_PAD:3 * R_PAD]
        dxn = bc[0:PTS, 3 * R_PAD:4 * R_PAD]

        def ptile(nm, dt_=None):
            return prep_pool.tile([PTS, R_PAD], dt_ or f32, tag=nm, name=nm)

        def prep_axis(ax, gcol, lo_n, d_n, hdim):
            """returns (c0f, cbf, lc, mc) tiles [49, R_PAD] f32"""
            inn = ptile(f"inn{ax}")
            nc.vector.scalar_tensor_tensor(inn[:, :], d_n, gcol[:, :], lo_n,
                                           Alu.mult, Alu.add)
            nc.vector.tensor_scalar_mul(inn[:, :], inn[:, :], hdim - 1.0)
            cc = ptile(f"cc{ax}")
            nc.vector.tensor_scalar(cc[:, :], inn[:, :], 0.0, hdim - 1.0,
                                    Alu.max, Alu.min)
            # exact floor for 0<=x<2^22: t=(x+2^23)-2^23 is round-nearest;
            # subtract 1 where t > x
            rnd = ptile(f"rnd{ax}")
            nc.vector.tensor_scalar(rnd[:, :], cc[:, :], 8388608.0, 8388608.0,
                                    Alu.add, Alu.subtract)
            gt = ptile(f"gt{ax}")
            nc.vector.tensor_tensor(gt[:, :], rnd[:, :], cc[:, :], Alu.is_gt)
            c0f = ptile(f"c0f{ax}")
            nc.vector.tensor_sub(c0f[:, :], rnd[:, :], gt[:, :])
            cbf = ptile(f"cbf{ax}")
            nc.vector.tensor_scalar(cbf[:, :], c0f[:, :], 1.0, hdim - 1.0,
                                    Alu.add, Alu.min)
            lc = ptile(f"lc{ax}")
            nc.vector.tensor_sub(lc[:, :], inn[:, :], c0f[:, :])
            m1 = ptile(f"m1{ax}")
            nc.vector.tensor_scalar(m1[:, :], inn[:, :], 0.0, None, Alu.is_ge)
            m2 = ptile(f"m2{ax}")
            nc.vector.tensor_scalar(m2[:, :], inn[:, :], hdim - 1.0, None,
                                    Alu.is_le)
            mc = ptile(f"mc{ax}")
            nc.vector.tensor_mul(mc[:, :], m1[:, :], m2[:, :])
            return inn, c0f, cbf, lc, mc

        _, y0f, ybf, ly, my = prep_axis("y", gy_col, y1n, dyn, float(H))
        _, x0f, xbf, lx, mx = prep_axis("x", gx_col, x1n, dxn, float(W))

        def ab(ax_, lc, mc):
            a = ptile(f"a{ax_}")
            nc.vector.tensor_scalar(a[:, :], lc[:, :], -1.0, 1.0, Alu.mult,
                                    Alu.add)
            nc.vector.tensor_mul(a[:, :], a[:, :], mc[:, :])
            b = ptile(f"b{ax_}")
            nc.vector.tensor_mul(b[:, :], lc[:, :], mc[:, :])
            return a, b

        ay, by = ab("y", ly, my)
        ax, bx = ab("x", lx, mx)

        # per-corner weights (fp16) and indices (int16)
        corners = []  # (w16 tile, idx16 tile)
        for kc, (wy, wx_, yf, xf) in enumerate(
                ((ay, ax, y0f, x0f), (ay, bx, y0f, xbf),
                 (by, ax, ybf, x0f), (by, bx, ybf, xbf))):
            w16 = ptile(f"w16_{kc}", f16)
            nc.vector.tensor_mul(w16[:, :], wy[:, :], wx_[:, :])
            idxf = ptile(f"idxf{kc}")
            nc.vector.scalar_tensor_tensor(idxf[:, :], yf[:, :], float(W),
                                           xf[:, :], Alu.mult, Alu.add)
            idx16 = ptile(f"idx16_{kc}", i16)
            nc.vector.tensor_copy(idx16[:, :], idxf[:, :])
            corners.append((w16, idx16))

        # ------------------------------------------------ idx wrap + W flatten
        # gather order within batch b: j = rl*64 + pt  (rl<16, pt<49 valid)
        # wrapped: partition p = pt%16 (q=pt//16<4), slot s = rl*4 + q
        # idxw[k] free layout: [b(10), s(64)]
        idxw = const_pool.tile([128, 4, NBATCH, SPB], i16, tag="idxw")
        nc.gpsimd.memset(idxw[:, :, :, :], 0)
        # wflat: partition k holds corner k's flat row [b(10), rl(16), pt-slot(64)]
        wdram = dram_pool.tile([4, NBATCH * NIDX], f16, name="wdram")
        for k, (w16, idx16) in enumerate(corners):
            # idx wrap: dst[p, k, b, rl*4+q] = idx16[q*16+p, b*16+rl]
            for q in range(4):
                npq = min(16, PTS - q * 16)  # 16,16,16,1
                src = idx16[q * 16:q * 16 + npq, :].rearrange(
                    "p (b r) -> p b r", b=NBATCH)
                dst = idxw[0:npq, k, :, :].rearrange(
                    "p b (r q) -> p b r q", q=4)[:, :, :, q]
                nc.sync.dma_start(dst, src)
            # w flatten: wflat[k, b*1024 + rl*64 + pt] = w16[pt, b*16+rl]
            # dst iterated (s, b, r) to match src element order (p, b, r)
            dstw = wdram[k:k + 1, :].rearrange(
                "o (b r s) -> o s b r", b=NBATCH, r=RB)[:, 0:PTS, :, :]
            nc.sync.dma_start(dstw, w16[:, :].rearrange(
                "p (b r) -> p b r", b=NBATCH))
        for k in range(3):
            lo, hi = 16 << k, 32 << k
            nc.sync.dma_start(idxw[lo:hi, :, :, :], idxw[0:hi - lo, :, :, :])

        # ------------------------------------------------ main loop
        for b in range(NBATCH):
            # rois beyond R_CORE are host-side padding; skip fully-pad batches
            nv = RB if (b + 1) * RB <= R_CORE else R_CORE - b * RB
            if nv <= 0:
                continue
            ob = o_pool.tile([128, 4, RB, PTS], f16, tag="O")
            for k in range(4):
                tk = gather_pool.tile([128, 4, NIDX], f16, tag="T")
                nc.gpsimd.dma_gather(
                    tk[:, :, :], img_g[:, :, :], idxw[:, k, b, :],
                    NIDX, NIDX, C,
                    transpose=True,
                    sbuf_tokens_per_rank=128,
                    sbuf_free_dim_per_rank=C * 2,
                )
                wrow = wrow_pool.tile([1, NIDX], f16, tag="wr")
                nc.sync.dma_start(wrow[:, :],
                                  wdram[k:k + 1, b * NIDX:(b + 1) * NIDX])
                wk = w_pool.tile([128, NIDX], f16, tag="W")
                ps = psum_pool.tile([128, NIDX], f32, tag="ps")
                nc.tensor.matmul(ps[:, :], ones16[:, :], wrow[:, :],
                                 start=True, stop=True)
                nc.scalar.copy(wk[:, :], ps[:, :])
                # valid-slot views [128, 4, RB, PTS]
                tv = tk[:, :, :].rearrange("p e (r s) -> p e r s",
                                           r=RB)[:, :, :, 0:PTS]
                wv = wk[:, :].rearrange("p (r s) -> p r s",
                                        r=RB)[:, :, 0:PTS]
                wv4 = wv  # broadcast over e by explicit per-e ops
                if k == 0:
                    for e in range(4):
                        nc.vector.tensor_mul(ob[:, e, :, :], tv[:, e, :, :],
                                             wv4)
                else:
                    for e in range(4):
                        nc.vector.tensor_mul(tv[:, e, :, :], tv[:, e, :, :],
                                             wv4)
                    nc.vector.tensor_add(ob[:, :, :, :], ob[:, :, :, :], tv)

            # cast fp16 -> f32 on Activation, then HWDGE write (no SWDGE gen)
            ob32 = o_pool.tile([128, 4, RB, PTS], f32, tag="O32")
            nc.scalar.copy(ob32[:, :, :, :], ob[:, :, :, :])
            dste = out_t.ap()[b * RB:b * RB + nv, :, :, :].rearrange(
                "r (e p) py px -> p e r (py px)", e=4)
            for e in range(4):
                nc.sync.dma_start(dste[:, e, :, :], ob32[:, e, 0:nv, :])


def _get_program():
    if "nc" not in _prog_cache:
        _prog_cache["nc"] = _build_program()
    return _prog_cache["nc"]


def _make_consts():
    consts = np.zeros((1, 256), dtype=np.float32)
    g = (np.arange(POOL, dtype=np.float32) / np.float32(POOL - 1.0)).astype(
        np.float32)
    gy = np.repeat(g, POOL)   # g[pt//7]
    gx = np.tile(g, POOL)     # g[pt%7]
    consts[0, 0:PTS] = gy
    consts[0, PTS:2 * PTS] = gx
    consts[0, 98:226] = 1.0
    return consts


def kernel(img: np.ndarray, rois: np.ndarray,
           input_image: np.ndarray) -> np.ndarray:
    from concourse.bass_utils import run_bass_kernel_spmd

    nc = _get_program()
    consts = _make_consts()
    in_maps = []
    img16 = {}
    for n in range(N):
        img16[n] = np.ascontiguousarray(
            img[n].reshape(C, HW).T.astype(np.float16))
    for c in range(N_CORES):
        n, half = c // 2, c % 2
        rpad = np.zeros((R_PAD, 4), dtype=np.float32)
        rpad[:R_CORE] = rois[n, half * R_CORE:(half + 1) * R_CORE]
        in_maps.append({
            "img": img16[n],
            "rois": rpad,
            "consts": consts,
        })
    res = run_bass_kernel_spmd(nc, in_maps, core_ids=list(range(N_CORES)))
    out = np.empty((N, B, C, POOL, POOL), dtype=np.float32)
    for c in range(N_CORES):
        n, half = c // 2, c % 2
        out[n, half * R_CORE:(half + 1) * R_CORE] = res.results[c]["out"]
    return out

